# revision 15
# baseline (speedup 1.0000x reference)
"""Trainium2 Bass kernel for nn_NNModel_35356170780677.

Spiking RNN: embedding gather -> 2-layer rhythmic-masked recurrence (T=128)
-> vocab decode [4096,512]@[512,32000].

Sharding: recurrence replicated on all 8 cores; decoder vocab dim (32000)
split 8 x 4000. Embedding gather + transposes + fp22/fp8 packing on host.

Fast (all-ones masks) pipeline per core, chunked by 8 timesteps:
  - state1 = embT @ fc1 via host-split fp22 hi/lo f32r matmuls
  - state2 = gates @ (-fc2) f32r (hi/lo or single) + colsum bias
  - serial membrane chain on DVE: 2 fused ops/step
      tmp = (mem<=TH)*mem ; mem' = tmp*DECAY + state
  - gates/spikes batch-extracted per chunk on GpSimd straight from the
    membrane store (spikes2 written as fp8 in DoubleRow lhsT layout)
  - decode: fp8 DoubleRow matmuls (2 MACs/cell/cycle), PSUM drained by
    ACT/DVE into a bf16 row buffer, one 1MB DMA per 128-row block
General (rhythmic masks) path: previous proven implementation, unchanged.
"""

import os
import sys
import types
import numpy as np
import ml_dtypes
from contextlib import ExitStack

import concourse.bass as bass
import concourse.tile as tile
import concourse.bacc as bacc
from concourse import mybir
from concourse.bass_utils import run_bass_kernel_spmd

F32 = mybir.dt.float32
F32R = mybir.dt.float32r
BF16 = mybir.dt.bfloat16
FP8 = mybir.dt.float8e4
ALU = mybir.AluOpType
AFT = mybir.ActivationFunctionType
DR = mybir.MatmulPerfMode.DoubleRow

T, B, NTOK, NINP, H1, H2 = 128, 32, 32000, 256, 512, 512
NCORES = 8
VSH = NTOK // NCORES            # 4000 vocab per core
TB = T * B                      # 4096
LAG = 16                        # layer2 lags layer1 by LAG iters
ITERS = T + LAG                 # 144
CH = 8                          # iters per chunk
NCHUNK = ITERS // CH            # 18
TH = 0.6
DECAY = 0.6
N_TILES = [(i * 512, min(512, VSH - i * 512)) for i in range((VSH + 511) // 512)]

# precision knobs for the fast path
S1_TERMS = 2      # 3: fhi*ehi+flo*ehi+fhi*elo, 2: drop fhi*elo, 1: raw f32r
S2_TERMS = 1      # 2: fc2 hi/lo, 1: raw f32r
# decode psum drain engine per group (8 groups/chunk): 'a'=ACT, 'v'=DVE
DEC_COPY_PAT = "aaaavaaa"

# module-level knobs / results (used by test harness)
TRACE = False
LAST_EXEC_NS = None
LAST_TRACE_PATH = None
_BUILT = {}
LDW_OPT = False


def _enable_ldw_opt():
    """Compile with walrus --enable-ldw-opt=true (elides redundant
    LDWEIGHTS when consecutive matmuls share the stationary operand)."""
    import concourse.bass_utils as bu
    if getattr(bu, "_ldw_wrapper", None):
        return
    real = bu.get_walrus_driver()
    wrapper = "/tmp/walrus_ldw_wrapper.py"
    with open(wrapper, "w") as f:
        f.write(
            "#!/usr/bin/env python3\n"
            "import os, sys\n"
            f"real = {real!r}\n"
            "args = [a.replace('--enable-ldw-opt=false',"
            " '--enable-ldw-opt=true') for a in sys.argv[1:]]\n"
            "os.execv(real, [real] + args)\n")
    import stat
    os.chmod(wrapper, os.stat(wrapper).st_mode | stat.S_IEXEC)
    bu.get_walrus_driver = lambda: wrapper
    bu._ldw_wrapper = wrapper


def _install_ntff_hook():
    """Register the NTFF profile hook that the image's antenv lacks."""
    if "antenv.axon_hooks" in sys.modules:
        return
    try:
        import antenv
        mod = types.ModuleType("antenv.axon_hooks")
        mod._hook = None
        mod.set_axon_ntff_profile_hook = lambda h: setattr(mod, "_hook", h)
        mod.get_axon_ntff_profile_hook = lambda: mod._hook
        sys.modules["antenv.axon_hooks"] = mod
        antenv.axon_hooks = mod
        from trn_agent_boot.trn_boot import _ntff_profile_via_ctypes
        mod._hook = _ntff_profile_via_ctypes("/opt/axon/libaxon_pjrt.so")
        import concourse.bass_utils as bu
        bu.upload_artifacts = lambda tmpdir: f"local://{tmpdir}"
    except Exception:
        pass


def build_fast(s1_terms: int, s2_terms: int):
    nc = bacc.Bacc("TRN2", target_bir_lowering=False, debug=False,
                   enable_asserts=True, num_devices=NCORES)
    ehi_d = nc.dram_tensor("ehi", [NINP, TB], F32R, kind="ExternalInput").ap()
    if s1_terms >= 3:
        elo_d = nc.dram_tensor("elo", [NINP, TB], F32R, kind="ExternalInput").ap()
    f1h_d = nc.dram_tensor("f1h", [NINP, H1], F32R, kind="ExternalInput").ap()
    if s1_terms >= 2:
        f1l_d = nc.dram_tensor("f1l", [NINP, H1], F32R, kind="ExternalInput").ap()
    f2h_d = nc.dram_tensor("f2h", [H1, H2], F32R, kind="ExternalInput").ap()
    if s2_terms >= 2:
        f2l_d = nc.dram_tensor("f2l", [H1, H2], F32R, kind="ExternalInput").ap()
    bias_d = nc.dram_tensor("bias", [128, 4], F32, kind="ExternalInput").ap()
    decw_d = nc.dram_tensor("decw8", [128, 4 * VSH], FP8, kind="ExternalInput").ap()
    out_d = nc.dram_tensor("out", [TB, VSH], BF16, kind="ExternalOutput").ap()

    with tile.TileContext(nc, trace_sim=False) as tc:
        with ExitStack() as ctx:
            wp = ctx.enter_context(tc.tile_pool(name="weights", bufs=1))
            embp = ctx.enter_context(tc.tile_pool(name="embp", bufs=7))
            scatp = ctx.enter_context(tc.tile_pool(name="scatp", bufs=7))
            mcatp = ctx.enter_context(tc.tile_pool(name="mcatp", bufs=3))
            g1p = ctx.enter_context(tc.tile_pool(name="g1p", bufs=3))
            z8p = ctx.enter_context(tc.tile_pool(name="z8p", bufs=3))
            obp = ctx.enter_context(tc.tile_pool(name="obp", bufs=2))
            tmpp = ctx.enter_context(tc.tile_pool(name="tmpp", bufs=2))
            psp = ctx.enter_context(tc.tile_pool(name="psp", bufs=2, space="PSUM"))
            pdp = ctx.enter_context(tc.tile_pool(name="pdp", bufs=3, space="PSUM"))

            # ---- resident weights (decoder last: pipeline starts sooner) ----
            fc1h, fc1l = [], []
            for kt in range(2):
                t_ = wp.tile([128, H1], F32R, tag=f"f1h_{kt}")
                nc.sync.dma_start(t_[:], f1h_d[kt * 128:(kt + 1) * 128, :])
                fc1h.append(t_)
                if s1_terms >= 2:
                    t2_ = wp.tile([128, H1], F32R, tag=f"f1l_{kt}")
                    nc.sync.dma_start(t2_[:], f1l_d[kt * 128:(kt + 1) * 128, :])
                    fc1l.append(t2_)
            fc2h, fc2l = [], []
            for j in range(4):
                t_ = wp.tile([128, H2], F32R, tag=f"f2h_{j}")
                nc.sync.dma_start(t_[:], f2h_d[j * 128:(j + 1) * 128, :])
                fc2h.append(t_)
                if s2_terms >= 2:
                    t2_ = wp.tile([128, H2], F32R, tag=f"f2l_{j}")
                    nc.sync.dma_start(t2_[:], f2l_d[j * 128:(j + 1) * 128, :])
                    fc2l.append(t2_)
            bias_sb = wp.tile([128, 4], F32, tag="bias")
            nc.sync.dma_start(bias_sb[:], bias_d)
            mem_init = wp.tile([128, 256], F32, tag="mem_init")
            nc.gpsimd.memset(mem_init[:], 0.0)
            decw_sb = wp.tile([128, 4 * VSH], FP8, tag="decw8")
            nc.sync.dma_start(decw_sb[:], decw_d)
            w4 = decw_sb[:].rearrange("p (jj kk v) -> p jj kk v", jj=2, kk=2)

            embt = {}     # chunk -> list of (hi tiles, lo tiles) per kt
            scat = {}     # chunk -> [128, 2048] f32 state cat (s8, h2, j4, b32)
            mcat = {}     # chunk -> [128, 2048] f32 membrane cat (same layout)
            g1r = {}      # chunk -> [128, 1024] f32 {0,1} gates1, (j4, s8, b32)
            z8 = {}       # chunk -> [128, 1024] fp8 spikes2, (mt2, jj2, kk2, m128)

            def dma_embt(ec):
                tiles = []
                for kt in range(2):
                    th_ = embp.tile([128, 256], F32R, tag=f"eh_{kt}")
                    nc.sync.dma_start(
                        th_[:], ehi_d[kt * 128:(kt + 1) * 128,
                                      ec * 256:(ec + 1) * 256])
                    if s1_terms >= 3:
                        tl_ = embp.tile([128, 256], F32R, tag=f"el_{kt}")
                        nc.sync.dma_start(
                            tl_[:], elo_d[kt * 128:(kt + 1) * 128,
                                          ec * 256:(ec + 1) * 256])
                    else:
                        tl_ = None
                    tiles.append((th_, tl_))
                embt[ec] = tiles

            def ensure_scat_l1(ec):
                """Allocate chunk ec's state store + fill its layer1 half."""
                if ec in scat:
                    return
                st = scatp.tile([128, 2048], F32, tag="scat")
                scat[ec] = st
                st5 = st[:].rearrange("p (s h j b) -> p s h j b",
                                      s=8, h=2, j=4, b=32)
                if ec <= 1 or ec >= 16:
                    nc.gpsimd.memset(st[:], 0.0)
                if ec > 15:
                    return
                # terms: (weight split, emb split idx) pairs
                terms = [(fc1h, 0)]
                if s1_terms >= 2:
                    terms.append((fc1l, 0))
                if s1_terms >= 3:
                    terms.append((fc1h, 1))
                for jp in range(2):          # j pairs (0,1) and (2,3)
                    ps = psp.tile([128, 512], F32, tag="ps_s")
                    for jo in range(2):
                        j = jp * 2 + jo
                        first = True
                        for ti, (wsp, esp) in enumerate(terms):
                            last_t = ti == len(terms) - 1
                            for kt in range(2):
                                emb_ap = embt[ec][kt][esp]
                                nc.tensor.matmul(
                                    ps[:, jo * 256:(jo + 1) * 256],
                                    wsp[kt][:, j * 128:(j + 1) * 128],
                                    emb_ap[:],
                                    start=first, stop=(last_t and kt == 1))
                                first = False
                    # drain pair: src [jo2, s8, b32] -> dst (s, h=0, j=jp*2+jo, b)
                    dst = st5[:, :, 0, jp * 2:jp * 2 + 2, :]
                    src = ps[:].rearrange("p (jo s b) -> p s jo b", jo=2, s=8)
                    nc.scalar.copy(dst, src)

            for ec0 in range(5):
                dma_embt(ec0)
            for ec0 in range(4):
                ensure_scat_l1(ec0)

            mem_prev_ap = mem_init[:]

            for ic in range(-1, NCHUNK + 1):
                # ---- prefetch emb ----
                if 5 <= ic + 5 <= 15 and (ic + 5) not in embt:
                    dma_embt(ic + 5)

                # ---- build state_cat for chunk ec = ic+1 ----
                ec = ic + 1
                if 0 <= ec <= NCHUNK - 1:
                    ensure_scat_l1(ec)
                    if ec + 1 <= NCHUNK - 1:
                        ensure_scat_l1(ec + 1)
                    st5 = scat[ec][:].rearrange("p (s h j b) -> p s h j b",
                                                s=8, h=2, j=4, b=32)
                    if ec >= 2:
                        grt = g1r[ec - 2]
                        g4 = grt[:].rearrange("p (s j b) -> p j s b", s=8, j=4)
                        splits = [fc2h] + ([fc2l] if s2_terms >= 2 else [])
                        for ibp in range(2):     # ib pairs
                            ps = psp.tile([128, 512], F32, tag="ps_s")
                            for io in range(2):
                                ib = ibp * 2 + io
                                for j in range(4):
                                    for si, sp in enumerate(splits):
                                        nc.tensor.matmul(
                                            ps[:, io * 256:(io + 1) * 256],
                                            sp[j][:, ib * 128:(ib + 1) * 128],
                                            g4[:, j],
                                            start=(j == 0 and si == 0),
                                            stop=(j == 3 and
                                                  si == len(splits) - 1))
                                # bias is per-ib -> per-half activation copy
                                dst = st5[:, :, 1, ib, :]
                                src = ps[:, io * 256:(io + 1) * 256].rearrange(
                                    "p (s b) -> p s b", s=8)
                                nc.scalar.activation(
                                    dst, src, AFT.Identity,
                                    bias=bias_sb[:, ib:ib + 1], scale=1.0)

                # ---- decode chunk dc = ic-1 (z8[dc] extracted last iter) ----
                dc = ic - 1
                if 2 <= dc <= NCHUNK - 1:
                    zv = z8[dc][:].rearrange(
                        "p (mt jj kk m) -> p mt jj kk m",
                        mt=2, jj=2, kk=2)
                    row0 = 256 * (dc - 2)
                    ci = 0
                    for mt in range(2):
                        ob = obp.tile([128, VSH], BF16, tag=f"ob_{mt}")
                        for g in range(4):
                            nts = N_TILES[2 * g:2 * g + 2]
                            psd = pdp.tile([128, 1024], F32, tag="ps_d")
                            for jj in range(2):
                                for ni, (noff, nsz) in enumerate(nts):
                                    nc.tensor.matmul(
                                        psd[:, ni * 512:ni * 512 + nsz],
                                        zv[:, mt, jj],
                                        w4[:, jj, :, noff:noff + nsz],
                                        start=(jj == 0), stop=(jj == 1),
                                        perf_mode=DR,
                                        skip_group_check=True)
                            gw = nts[-1][0] + nts[-1][1] - nts[0][0]
                            eng = DEC_COPY_PAT[ci % len(DEC_COPY_PAT)]
                            ci += 1
                            if gw == 1024:
                                obs = ob[:, nts[0][0]:nts[0][0] + 1024]
                                if eng == 'a':
                                    nc.scalar.copy(obs, psd[:])
                                else:
                                    nc.vector.tensor_copy(obs, psd[:])
                            else:
                                # last group: psum halves 0:512 and 512:512+416
                                p0 = psd[:, 0:512]
                                p1 = psd[:, 512:512 + nts[1][1]]
                                o0 = ob[:, nts[0][0]:nts[0][0] + 512]
                                o1 = ob[:, nts[1][0]:nts[1][0] + nts[1][1]]
                                if eng == 'a':
                                    nc.scalar.copy(o0, p0)
                                    nc.scalar.copy(o1, p1)
                                else:
                                    nc.vector.tensor_copy(o0, p0)
                                    nc.vector.tensor_copy(o1, p1)
                        nc.sync.dma_start(
                            out_d[row0 + mt * 128:row0 + (mt + 1) * 128, :],
                            ob[:])

                # ---- serial chain for chunk ic ----
                if 0 <= ic <= NCHUNK - 1:
                    mct = mcatp.tile([128, 2048], F32, tag="mcat")
                    mcat[ic] = mct
                    for s in range(CH):
                        state_ap = scat[ic][:, s * 256:(s + 1) * 256]
                        tmp = tmpp.tile([128, 256], F32, tag="tmp")
                        nc.vector.scalar_tensor_tensor(
                            tmp[:], mem_prev_ap, TH,
                            mem_prev_ap, ALU.is_le, ALU.mult)
                        mem_ap = mct[:, s * 256:(s + 1) * 256]
                        nc.vector.scalar_tensor_tensor(
                            mem_ap, tmp[:], DECAY, state_ap,
                            ALU.mult, ALU.add)
                        mem_prev_ap = mem_ap
                    m5 = mct[:].rearrange("p (s h j b) -> p s h j b",
                                          s=8, h=2, j=4, b=32)
                    if ic <= 15:
                        # gates1 = (mem1 <= TH), packed (s, j, b) f32r
                        grt = g1p.tile([128, 1024], F32R, tag="g1r")
                        g1r[ic] = grt
                        gsrc = m5[:, :, 0, :, :]
                        gdst = grt[:].rearrange("p (s j b) -> p s j b",
                                                s=8, j=4)
                        nc.vector.tensor_scalar(gdst, gsrc, TH, None, ALU.is_le)
                    if ic >= 2:
                        # spikes2 = (mem2 > TH) fp8, DoubleRow lhsT layout
                        # [p, (mt2, jj2, kk2, m128)], m = (sl, b) contiguous
                        zt = z8p.tile([128, 1024], FP8, tag="z8")
                        z8[ic] = zt
                        zv = zt[:].rearrange("p (mt j m) -> p mt j m",
                                             mt=2, j=4, m=128)
                        for mt in range(2):
                            zsrc = m5[:, 4 * mt:4 * mt + 4, 1, :, :].rearrange(
                                "p sl j b -> p j sl b")
                            zdst = zv[:, mt].rearrange(
                                "p j (sl b) -> p j sl b", sl=4)
                            nc.vector.tensor_scalar(zdst, zsrc, TH, None,
                                                    ALU.is_gt)

    nc.compile()
    return nc


def build(general: bool):
    """General-mask (rhythmic) path — unchanged proven implementation."""
    assert general
    nc = bacc.Bacc("TRN2", target_bir_lowering=False, debug=False,
                   enable_asserts=True, num_devices=NCORES)
    embT_d = nc.dram_tensor("embT", [NINP, TB], F32, kind="ExternalInput").ap()
    fc1_d = nc.dram_tensor("fc1", [NINP, H1], F32, kind="ExternalInput").ap()
    fc2_d = nc.dram_tensor("fc2e", [H1, H2], F32, kind="ExternalInput").ap()
    bias_d = nc.dram_tensor("bias", [128, 4], F32, kind="ExternalInput").ap()
    decw_d = nc.dram_tensor("decwT", [H2, VSH], BF16, kind="ExternalInput").ap()
    mcat_d = nc.dram_tensor("mcat", [128, ITERS * 256], F32,
                            kind="ExternalInput").ap()
    mbcat_d = nc.dram_tensor("mbcat", [128, ITERS * 256], F32,
                             kind="ExternalInput").ap()
    out_d = nc.dram_tensor("out", [TB, VSH], F32, kind="ExternalOutput").ap()

    with tile.TileContext(nc, trace_sim=False) as tc:
        with ExitStack() as ctx:
            wp = ctx.enter_context(tc.tile_pool(name="weights", bufs=1))
            tmp = ctx.enter_context(tc.tile_pool(name="tmp", bufs=1))
            embp = ctx.enter_context(tc.tile_pool(name="embp", bufs=3))
            scatp = ctx.enter_context(tc.tile_pool(name="scatp", bufs=2))
            gp = ctx.enter_context(tc.tile_pool(name="gp", bufs=2))
            zp = ctx.enter_context(tc.tile_pool(name="zp", bufs=3))
            memp = ctx.enter_context(tc.tile_pool(name="memp", bufs=3))
            up = ctx.enter_context(tc.tile_pool(name="up", bufs=2))
            obp = ctx.enter_context(tc.tile_pool(name="obp", bufs=6))
            ps1p = ctx.enter_context(tc.tile_pool(name="ps1p", bufs=2, space="PSUM"))
            ps2p = ctx.enter_context(tc.tile_pool(name="ps2p", bufs=2, space="PSUM"))
            pdp = ctx.enter_context(tc.tile_pool(name="pdp", bufs=4, space="PSUM"))
            mp = ctx.enter_context(tc.tile_pool(name="mp", bufs=2))

            fc1_sb = []
            for kt in range(2):
                t_ = wp.tile([128, H1], F32, tag=f"fc1_{kt}")
                nc.sync.dma_start(t_[:], fc1_d[kt * 128:(kt + 1) * 128, :])
                fc1_sb.append(t_)
            g_init = wp.tile([128, 256], F32, tag="g_init")
            nc.gpsimd.memset(g_init[:], 1.0)
            mem_init = wp.tile([128, 256], F32, tag="mem_init")
            nc.gpsimd.memset(mem_init[:], 0.0)

            scat = {}
            g01 = {}
            zms = {}
            g1r = {}
            z2c = {}
            z2s = {}
            embt = {}
            mca = {}
            mba = {}

            def dma_embt(ec):
                tiles = []
                for kt in range(2):
                    t_ = embp.tile([128, 256], F32, tag=f"embt_{kt}")
                    nc.sync.dma_start(
                        t_[:], embT_d[kt * 128:(kt + 1) * 128,
                                      ec * 256:(ec + 1) * 256])
                    tiles.append(t_)
                embt[ec] = tiles

            def dma_masks(mc):
                mt_ = mp.tile([128, 2048], F32, tag="mcat")
                nc.sync.dma_start(mt_[:], mcat_d[:, mc * 2048:(mc + 1) * 2048])
                mca[mc] = mt_
                bt_ = mp.tile([128, 2048], F32, tag="mbcat")
                nc.sync.dma_start(bt_[:], mbcat_d[:, mc * 2048:(mc + 1) * 2048])
                mba[mc] = bt_

            for ec0 in range(2):
                dma_embt(ec0)
            dma_masks(0)
            fc2_hi, fc2_lo = [], []
            for j in range(4):
                raw = tmp.tile([128, H2], F32, tag="fc2raw")
                nc.sync.dma_start(raw[:], fc2_d[j * 128:(j + 1) * 128, :])
                hi = wp.tile([128, H2], F32R, tag=f"fc2hi_{j}")
                nc.vector.tensor_copy(hi[:], raw[:])
                diff = tmp.tile([128, H2], F32, tag="fc2diff")
                nc.vector.tensor_tensor(diff[:], raw[:],
                                        hi[:].bitcast(F32), ALU.subtract)
                lo = wp.tile([128, H2], F32R, tag=f"fc2lo_{j}")
                nc.vector.tensor_copy(lo[:], diff[:])
                fc2_hi.append(hi)
                fc2_lo.append(lo)
            bias_sb = wp.tile([128, 4], F32, tag="bias")
            nc.sync.dma_start(bias_sb[:], bias_d)
            decw_sb = []
            for j in range(4):
                t_ = wp.tile([128, VSH], BF16, tag=f"decw_{j}")
                nc.sync.dma_start(t_[:], decw_d[j * 128:(j + 1) * 128, :])
                decw_sb.append(t_)

            def ensure_scat_l1(ec):
                if ec in scat:
                    return
                st = scatp.tile([128, 2048], F32, tag="scat")
                scat[ec] = st
                st5 = st[:].rearrange("p (s h j b) -> p s h j b",
                                      s=8, h=2, j=4, b=32)
                if ec <= 1 or ec >= 16:
                    nc.gpsimd.memset(st[:], 0.0)
                if ec <= 15:
                    for j in range(4):
                        ps = ps1p.tile([128, 256], F32, tag="ps1")
                        nc.tensor.matmul(
                            ps[:], fc1_sb[0][:, j * 128:(j + 1) * 128],
                            embt[ec][0][:], start=True, stop=False)
                        nc.tensor.matmul(
                            ps[:], fc1_sb[1][:, j * 128:(j + 1) * 128],
                            embt[ec][1][:], start=False, stop=True)
                        dst = st5[:, :, 0, j, :]
                        src = ps[:].rearrange("p (s b) -> p s b", s=8)
                        nc.scalar.copy(dst, src)

            mem_prev = mem_init
            gate_prev_ap = g_init[:]

            for ic in range(-1, NCHUNK + 1):
                if 2 <= ic + 2 <= 15 and (ic + 2) not in embt:
                    dma_embt(ic + 2)
                if 0 <= ic + 1 <= NCHUNK - 1:
                    dma_masks(ic + 1)

                ec = ic + 1
                if 0 <= ec <= NCHUNK - 1:
                    ensure_scat_l1(ec)
                    if ec + 1 <= NCHUNK - 1:
                        ensure_scat_l1(ec + 1)
                    st = scat[ec]
                    st5 = st[:].rearrange("p (s h j b) -> p s h j b",
                                          s=8, h=2, j=4, b=32)
                    if ec >= 2:
                        gc = ec - 2
                        grt = g1r[gc]
                        for ib in range(4):
                            ps = ps2p.tile([128, 256], F32, tag="ps2")
                            for j in range(4):
                                for si, sp in enumerate((fc2_hi, fc2_lo)):
                                    nc.tensor.matmul(
                                        ps[:],
                                        sp[j][:, ib * 128:(ib + 1) * 128],
                                        grt[:, j * 256:(j + 1) * 256],
                                        start=(j == 0 and si == 0),
                                        stop=(j == 3 and si == 1))
                            dst = st5[:, :, 1, ib, :]
                            src = ps[:].rearrange("p (s b) -> p s b", s=8)
                            nc.scalar.copy(dst, src)

                if 0 <= ic <= NCHUNK - 1:
                    gt = gp.tile([128, 2048], F32, tag="g01")
                    g01[ic] = gt
                    zmt = gp.tile([128, 2048], F32, tag="zm")
                    zms[ic] = zmt
                    if ic >= 2:
                        z2ct = zp.tile([128, 1024], BF16, tag="z2c")
                        z2c[ic] = z2ct
                    for s in range(CH):
                        state_ap = scat[ic][:, s * 256:(s + 1) * 256]
                        mem_cur = memp.tile([128, 256], F32, tag="mem")
                        u = up.tile([128, 256], F32, tag="u")
                        nc.vector.scalar_tensor_tensor(
                            u[:], mem_prev[:], DECAY,
                            gate_prev_ap, ALU.mult, ALU.mult)
                        m_ap = mca[ic][:, s * 256:(s + 1) * 256]
                        mb_ap = mba[ic][:, s * 256:(s + 1) * 256]
                        new = up.tile([128, 256], F32, tag="new")
                        nc.vector.tensor_tensor(
                            new[:], u[:], state_ap, ALU.add)
                        d = up.tile([128, 256], F32, tag="d")
                        nc.vector.tensor_tensor(
                            d[:], new[:], mem_prev[:], ALU.subtract)
                        dm = up.tile([128, 256], F32, tag="dm")
                        nc.vector.tensor_tensor(dm[:], d[:], mb_ap, ALU.mult)
                        nc.vector.tensor_tensor(
                            mem_cur[:], dm[:], mem_prev[:], ALU.add)
                        nc.vector.scalar_tensor_tensor(
                            zmt[:, s * 256:(s + 1) * 256],
                            mem_cur[:], TH, m_ap, ALU.is_gt, ALU.mult)
                        if ic >= 2:
                            nc.scalar.copy(
                                z2ct[:, s * 128:(s + 1) * 128],
                                zmt[:, s * 256 + 128:(s + 1) * 256])
                        nc.vector.tensor_scalar(
                            gt[:, s * 256:(s + 1) * 256], mem_cur[:],
                            TH, None, ALU.is_le)
                        mem_prev = mem_cur
                        gate_prev_ap = gt[:, s * 256:(s + 1) * 256]
                    if ic <= 15:
                        grt = gp.tile([128, 1024], F32R, tag="g1r")
                        g1r[ic] = grt
                        src = zms[ic][:].rearrange(
                            "p (s h j b) -> p s h j b",
                            s=8, h=2, j=4, b=32)[:, :, 0, :, :]
                        dst = grt[:].rearrange(
                            "p (j s b) -> p s j b", j=4, s=8, b=32)
                        nc.vector.tensor_copy(dst, src)
                    if ic >= 2:
                        z2rt = zp.tile([128, 1024], BF16, tag="z2r")
                        z2s[ic] = z2rt
                        src = z2ct[:].rearrange(
                            "p (mt sl j b) -> p mt j sl b",
                            mt=2, sl=4, j=4, b=32)
                        dst = z2rt[:].rearrange(
                            "p (mt j sl b) -> p mt j sl b",
                            mt=2, j=4, sl=4, b=32)
                        for mt in range(2):
                            nc.scalar.copy(dst[:, mt], src[:, mt])

                if 3 <= ic <= NCHUNK:
                    zt = z2s[ic - 1]
                    row0 = 256 * (ic - 3)
                    for mt in range(2):
                        for (noff, nsz) in N_TILES:
                            ps = pdp.tile([128, 512], F32, tag="psdec")
                            for j in range(4):
                                nc.tensor.matmul(
                                    ps[:, :nsz],
                                    zt[:, mt * 512 + j * 128:
                                       mt * 512 + (j + 1) * 128],
                                    decw_sb[j][:, noff:noff + nsz],
                                    start=(j == 0), stop=(j == 3))
                            ob = obp.tile([128, 512], F32, tag="ob")
                            nc.any.tensor_copy(ob[:, :nsz], ps[:, :nsz])
                            nc.sync.dma_start(
                                out_d[row0 + mt * 128:row0 + (mt + 1) * 128,
                                      noff:noff + nsz],
                                ob[:, :nsz])
    nc.compile()
    return nc


def _get_built(key):
    if key not in _BUILT:
        if key == "general":
            _BUILT[key] = build(True)
        else:
            if LDW_OPT:
                _enable_ldw_opt()
            _BUILT[key] = build_fast(S1_TERMS, S2_TERMS)
    return _BUILT[key]


def _trunc22(x):
    """Truncate fp32 mantissa to 11 bits (the PE's fp22 streaming format)."""
    v = np.ascontiguousarray(x, np.float32).view(np.uint32) & np.uint32(
        0xFFFFF000)
    return v.view(np.float32)


def _make_mcat(m1, m2):
    """Iter-indexed replicated mask concat [128, ITERS*256]."""
    out = np.zeros((128, ITERS, 2, 4, 32), np.float32)

    def rep(m):  # [512, T] -> [128, T, 4, 32]
        r = m.reshape(4, 128, T).transpose(1, 2, 0)      # [128, T, 4]
        return np.repeat(r[:, :, :, None], 32, axis=3)

    out[:, :T, 0] = rep(m1)
    out[:, LAG:LAG + T, 1] = rep(m2)
    return np.ascontiguousarray(out.reshape(128, ITERS * 256))


def kernel(**inputs) -> np.ndarray:
    global LAST_EXEC_NS, LAST_TRACE_PATH
    _install_ntff_hook()

    raw = np.asarray(inputs["raw_input"])
    enc_w = np.asarray(inputs["enc_w"], np.float32)
    fc1 = np.asarray(inputs["fc1"], np.float32)
    fc2 = np.asarray(inputs["fc2"], np.float32)
    dec_w = np.asarray(inputs["dec_w"], np.float32)
    dec_b = np.asarray(inputs["dec_b"], np.float32)
    m1 = np.asarray(inputs["mask1"], np.float32)[:, :T]
    m2 = np.asarray(inputs["mask2"], np.float32)[:, :T]

    ones = bool(np.all(m1 == 1.0) and np.all(m2 == 1.0))

    emb = enc_w[raw.reshape(-1).astype(np.int64)]          # [TB, NINP]
    embT = np.ascontiguousarray(emb.T)                     # [NINP, TB]
    fc2_eff = np.ascontiguousarray(-fc2)
    colsum = fc2.sum(axis=0, dtype=np.float64).astype(np.float32)      # [H2]
    bias = np.ascontiguousarray(colsum.reshape(4, 128).T)              # [128,4]

    if ones:
        ehi = _trunc22(embT)
        elo = np.ascontiguousarray(embT - ehi)
        f1h = _trunc22(fc1)
        f1l = np.ascontiguousarray(fc1 - f1h)
        f2h = _trunc22(fc2_eff)
        f2l = np.ascontiguousarray(fc2_eff - f2h)
        wt = dec_w.T                                       # [512, 32000]
        in_maps = []
        for c in range(NCORES):
            wc = wt[:, c * VSH:(c + 1) * VSH]              # [512, VSH]
            w8 = np.ascontiguousarray(
                wc.reshape(2, 2, 128, VSH).transpose(2, 0, 1, 3)
                .reshape(128, 4 * VSH)).astype(ml_dtypes.float8_e4m3)
            m = {"ehi": ehi, "f1h": f1h, "f2h": f2h,
                 "bias": bias, "decw8": w8}
            if S1_TERMS >= 3:
                m["elo"] = elo
            if S1_TERMS >= 2:
                m["f1l"] = f1l
            if S2_TERMS >= 2:
                m["f2l"] = f2l
            in_maps.append(m)
        nc = _get_built("fast")
    else:
        decwT = np.ascontiguousarray(dec_w.T).astype(ml_dtypes.bfloat16)
        mcat = _make_mcat(m1, m2)
        mbcat = (mcat != 0).astype(np.float32)
        in_maps = []
        for c in range(NCORES):
            m = {
                "embT": embT,
                "fc1": np.ascontiguousarray(fc1),
                "fc2e": fc2_eff,
                "bias": bias,
                "decwT": np.ascontiguousarray(decwT[:, c * VSH:(c + 1) * VSH]),
                "mcat": mcat,
                "mbcat": mbcat,
            }
            in_maps.append(m)
        nc = _get_built("general")

    res = run_bass_kernel_spmd(nc, in_maps, list(range(NCORES)), trace=TRACE)
    LAST_EXEC_NS = res.exec_time_ns
    if res.instructions_and_trace is not None:
        LAST_TRACE_PATH = res.instructions_and_trace[1]

    out = np.concatenate(
        [np.asarray(res.results[c]["out"], np.float32)
         for c in range(NCORES)], axis=1)
    if np.any(dec_b != 0.0):
        out = out + dec_b[None, :]
    return np.ascontiguousarray(out.reshape(T, B, NTOK), dtype=np.float32)
